# revision 1
# baseline (speedup 1.0000x reference)
"""Mixtral-style GQA attention (B=1, S=2048, HID=4096, 32 q-heads / 8 kv-heads,
head_dim=128, NeoX RoPE, causal) on 8 Trainium2 NeuronCores.

Sharding: tensor-parallel over heads. Core i gets q-heads [4i..4i+3] and
kv-head i (w_qkv columns), plus the matching w_o rows. Each core computes a
full-shape partial of the output projection; the host sums the 8 partials
(the "all-reduce") and returns the full output.

Device layout notes:
 - All matmuls run in fp32r (TF32) at 1 cycle/row; every tensor consumed by an
   fp32r matmul is produced only by f32r-writing instructions (walrus checks).
 - hidden_states is passed pre-transposed (XT [HID, S]) so the QKV projection
   needs no on-device transpose: qkvT[f, s] = sum_h W[h, f] * XT[h, s].
 - ALL DMAs issue from the otherwise-idle SP queue: DMA sequencer config
   costs ~600ns each and serializes against whatever engine queue carries
   it (exp dispatch on ACT, PSUM-eviction copies at window boundaries).
 - Phase 1 keeps all 32 W k-tiles resident (the qkv scratch is per-window);
   window 0 interleaves each weight pair with its X pair on the single DMA
   queue so it computes at stream pace.
 - Attention: the causal mask is computed INTO the score PSUM bank by PE
   itself (utri^T @ dmask_j, with the score matmul accumulating on top), so
   masking costs no cross-engine hop. Scores run one k-tile ahead of PV to
   hide the exp latency; softmax rowsums accumulate on DVE (even k-tiles)
   and Pool (odd) in SBUF, reduced+broadcast by one rank-128 matmul per
   head; normalization for head h is emitted during head h+1 with the
   o_proj block of a completed window interleaved to keep PE fed.
 - Softmax skips max-subtraction (scores are O(10), exp stays finite in
   f32). The last phase-1 window's RoPE is deferred into the attention
   phase (it only feeds the qw3 window) so its ops never sit in front of
   early softmax work on the in-order DVE/Pool queues.
 - o_proj PSUM eviction alternates DVE/ACT to split the copy load.
"""
from contextlib import ExitStack

import ml_dtypes
import numpy as np

import concourse.bacc as bacc
import concourse.tile as tile
from concourse import mybir
from concourse.bass_utils import run_bass_kernel_spmd

# ---- problem constants (hardcoded per contest contract) ----
HID = 4096
S = 2048
N_HEADS = 32
N_KV = 8
D = 128                    # head_dim
NCORES = 8
QH = N_HEADS // NCORES     # 4 q-heads per core
FEAT = QH * D + 2 * D      # 768 per-core qkv output columns (q0..q3, k, v)
FO = QH * D                # 512 per-core attn features for o_proj
ROPE_THETA = 10000.0
SCALE = D ** -0.5
MASK_NEG = -30000.0

P = 128
BF16 = mybir.dt.bfloat16
F32 = mybir.dt.float32
F32R = mybir.dt.float32r
EXP = mybir.ActivationFunctionType.Exp

NKT = HID // P     # 32 hidden k-tiles
NSW = S // 512     # 4 seq windows
NM = FEAT // P     # 6 qkv m-tiles
NST = S // P       # 16 seq tiles

_CACHE = {}
_MARKERS = []   # (label, approx I-id) appended during _kernel for trace reading


def _mark(nc, label):
    _MARKERS.append((label, nc.next_id()))


def _build_nc():
    nc = bacc.Bacc("TRN2", target_bir_lowering=False, debug=False)

    xt = nc.dram_tensor("xt", [HID, S], BF16, kind="ExternalInput").ap()
    wqkv = nc.dram_tensor("wqkv", [HID, FEAT], BF16, kind="ExternalInput").ap()
    wo = nc.dram_tensor("wo", [FO, HID], BF16, kind="ExternalInput").ap()
    cos_d = nc.dram_tensor("cos", [D, S], F32, kind="ExternalInput").ap()
    sinr_d = nc.dram_tensor("sinr", [D, S], F32, kind="ExternalInput").ap()
    utri_d = nc.dram_tensor("utri", [P, P], BF16, kind="ExternalInput").ap()
    dmask_d = nc.dram_tensor("dmask", [P, 4, 512], BF16, kind="ExternalInput").ap()
    onesm_d = nc.dram_tensor("ones_mat", [P, P], F32R, kind="ExternalInput").ap()
    id_d = nc.dram_tensor("ident", [P, P], F32, kind="ExternalInput").ap()
    out = nc.dram_tensor("out", [S, HID], F32, kind="ExternalOutput").ap()

    with tile.TileContext(nc) as tc:
        _kernel(tc, xt, wqkv, wo, cos_d, sinr_d, utri_d, dmask_d, onesm_d, id_d, out)
    nc.compile()
    return nc


def _kernel(tc, xt, wqkv, wo, cos_d, sinr_d, utri_d, dmask_d, onesm_d, id_d, out):
    nc = tc.nc

    with ExitStack() as big:
        persist = big.enter_context(tc.tile_pool(name="persist", bufs=1))
        wo_pool = big.enter_context(tc.tile_pool(name="wop", bufs=1))
        wo3 = wo.rearrange("(ft p) e -> p ft e", p=P)
        wo_sb = wo_pool.tile([P, QH, HID], BF16)
        onesm_sb = persist.tile([P, P], F32R)
        id_sb = persist.tile([P, P], F32)
        utri_sb = persist.tile([P, P], BF16)
        dmask_sb = persist.tile([P, 4, 512], BF16)
        roped = persist.tile([P, QH + 1, S], F32R)  # rotated q0..q3, K
        v_nat = persist.tile([P, NST, D], F32R)     # V in [seq-tile, dim] blocks
        # cos/sin and the LAST window's qkv scratch + rope temps live in the
        # persistent pool: phase-3 SBUF pools must never wait on the final
        # rope chunks (they run on the slow Pool engine into the attention
        # phase), so nothing phase 3 allocates may overlap what they read.
        cos_sb = persist.tile([D, S], F32)
        sinr_sb = persist.tile([D, S], F32)
        qkv_last = persist.tile([P, NM, 512], F32)
        rt_last = [persist.tile([P, 512], F32, name=f"rtl{i}")
                   for i in range(2)]

        deferred_rope = []
        deferred_tp = []

        def rope_chunk(qkv, m, sw, eng, rtmp=None):
            """roped[:,m,win] = qkv*cos + rot_half(qkv)*sin over a window."""
            win = slice(sw * 512, (sw + 1) * 512)
            row = qkv[:, m, :]
            if rtmp is None:
                tmp = rt_last[m % 2]
            else:
                tmp = rtmp.tile([P, 512], F32, tag="rt", name="ropetmp")
            eng.tensor_mul(tmp[0:64, :], row[64:128, :],
                           sinr_sb[64:128, win])
            eng.tensor_mul(tmp[64:128, :], row[0:64, :],
                           sinr_sb[0:64, win])
            eng.tensor_mul(row, row, cos_sb[:, win])
            eng.tensor_add(roped[:, m, win], row, tmp)

        # ---- phase 1: qkvT = wqkv^T @ XT, rope chunks interleaved ----
        wq3 = wqkv.rearrange("(kt p) f -> p kt f", p=P)
        with tc.tile_pool(name="qkvf", bufs=1) as qkvf_pool, \
             tc.tile_pool(name="rtmp", bufs=3) as rtmp, \
             tc.tile_pool(name="wq", bufs=1) as wq_pool, \
             tc.tile_pool(name="xts", bufs=8) as xt_pool:

            w_pair = [wq_pool.tile([P, 2, FEAT], BF16, tag=f"w{k}",
                                   name=f"w{k}") for k in range(NKT // 2)]
            w_res = [w_pair[kt // 2][:, kt % 2, :] for kt in range(NKT)]

            with tc.tile_pool(name="qkps", bufs=1, space="PSUM") as qk_ps, \
                 tc.tile_pool(name="tps", bufs=2, space="PSUM") as tp_ps:
                for sw in range(NSW):
                    _mark(nc, f"phase1_sw{sw}")
                    ps = [qk_ps.tile([P, 512], F32, tag=f"m{m}",
                                     name=f"qkps{m}") for m in range(NM)]
                    xt2 = xt.rearrange("(kt p) s -> p kt s", p=P)
                    for kt in range(NKT):
                        if kt % 2 == 0:
                            if sw == 0:
                                # single-queue issue order: each weight pair
                                # right before the X pair that joins it, so
                                # window 0 computes at stream pace. The very
                                # first pairs stream in halves: subtile deps
                                # let kt0 start after just the first halves.
                                if kt == 0:
                                    nc.sync.dma_start(
                                        out=w_pair[0][:, 0, :],
                                        in_=wq3[:, 0, :])
                                else:
                                    nc.sync.dma_start(
                                        out=w_pair[kt // 2],
                                        in_=wq3[:, kt:kt + 2, :])
                            xpair = xt_pool.tile([P, 2, 512], BF16,
                                                 name="xts")
                            if sw == 0 and kt == 0:
                                nc.sync.dma_start(out=xpair[:, 0, :],
                                                  in_=xt2[:, 0, 0:512])
                                nc.sync.dma_start(out=w_pair[0][:, 1, :],
                                                  in_=wq3[:, 1, :])
                                nc.sync.dma_start(out=xpair[:, 1, :],
                                                  in_=xt2[:, 1, 0:512])
                            else:
                                nc.sync.dma_start(
                                    out=xpair,
                                    in_=xt2[:, kt:kt + 2,
                                            sw * 512:(sw + 1) * 512])
                        xts = xpair[:, kt % 2, :]
                        for m in range(NM):
                            nc.tensor.matmul(
                                ps[m], w_res[kt][:, m * P:(m + 1) * P], xts,
                                start=(kt == 0), stop=(kt == NKT - 1))
                    if sw == 0:
                        # tables ride the ACT queue behind window-0's X, in
                        # chunks small enough to slip into the stream's
                        # spare DMA slots without stalling it.
                        nc.sync.dma_start(out=id_sb, in_=id_d)
                        for cch in range(4):
                            cw = slice(cch * 512, (cch + 1) * 512)
                            nc.sync.dma_start(out=cos_sb[:, cw],
                                                in_=cos_d[:, cw])
                            nc.sync.dma_start(out=sinr_sb[:, cw],
                                                in_=sinr_d[:, cw])
                    elif sw == 1:
                        nc.sync.dma_start(out=onesm_sb, in_=onesm_d)
                        nc.sync.dma_start(out=utri_sb, in_=utri_d)
                    elif sw == 2:
                        nc.sync.dma_start(out=dmask_sb, in_=dmask_d)
                        # prefetch w_o (bf16) while the DMA queue is light
                        for c in range(8):
                            cw = slice(c * 512, (c + 1) * 512)
                            nc.sync.dma_start(out=wo_sb[:, :, cw],
                                              in_=wo3[:, :, cw])
                    if sw == NSW - 1:
                        qkv = qkv_last
                    else:
                        qkv = qkvf_pool.tile([P, NM, 512], F32, tag="qkv",
                                             name="qkvwin")
                    # eviction order frees the banks the next window touches
                    # first; alternating ACT/DVE halves the eviction chain.
                    # V (m=5) evicts first so the transposes start at once.
                    nc.scalar.copy(out=qkv[:, QH + 1, :], in_=ps[QH + 1])
                    nc.vector.tensor_copy(out=qkv[:, 0, :], in_=ps[0])
                    nc.scalar.copy(out=qkv[:, 1, :], in_=ps[1])
                    nc.vector.tensor_copy(out=qkv[:, 2, :], in_=ps[2])
                    nc.scalar.copy(out=qkv[:, 3, :], in_=ps[3])
                    nc.vector.tensor_copy(out=qkv[:, 4, :], in_=ps[4])
                    # V transpose for this window's 4 seq blocks; the
                    # last window's are deferred into phase 3 (their v_nat
                    # tiles are only read by the qw3 attention window) so
                    # the phase transition never waits on them.
                    for j in range(4):
                        st = 4 * sw + j
                        if sw == NSW - 1:
                            deferred_tp.append((j, st))
                            continue
                        tp = tp_ps.tile([P, P], F32, tag="tp", name="tpps")
                        nc.tensor.transpose(
                            tp, qkv[:, QH + 1, j * P:(j + 1) * P], id_sb)
                        nc.scalar.copy(out=v_nat[:, st, :], in_=tp)
                    if sw == NSW - 1:
                        # defer: the last window's rope is only needed by the
                        # qw3 attention window, and its Pool/DVE ops would
                        # sit in front of the first heads' softmax work on
                        # the in-order queues. Emitted inside phase 3.
                        deferred_rope.extend(
                            (qkv, m, sw) for m in range(QH + 1))
                    else:
                        for m in range(QH + 1):   # rope, DVE/Pool split
                            rope_chunk(qkv, m, sw,
                                       eng=(nc.gpsimd if m % 2 == 0
                                            else nc.vector), rtmp=rtmp)

        # ---- phase 3: attention with o_proj fully interleaved ----
        kt_row = roped[:, QH, :]
        with tc.tile_pool(name="atn", bufs=1) as atn_pool, \
             tc.tile_pool(name="pts", bufs=10) as pt_pool, \
             tc.tile_pool(name="nsc", bufs=2) as norm_sc, \
             tc.tile_pool(name="ost", bufs=6) as o_stage, \
             tc.tile_pool(name="ops", bufs=1, space="PSUM") as o_ps, \
             tc.tile_pool(name="bcps", bufs=1, space="PSUM") as bc_ps:
            attnT = atn_pool.tile([P, QH, S], BF16)

            def oproj(st, pool, tags, mid_hook=None):
                _mark(nc, f"oproj_st{st}")
                ss = slice(st * P, (st + 1) * P)
                for ecg in range(4):
                    ops = [pool.tile([P, 512], F32,
                                     tag=tags[(ecg * 2 + ec) % len(tags)],
                                     name="opsb") for ec in range(2)]
                    for f in range(QH):
                        if ecg == 0 and f == QH - 1 and mid_hook is not None:
                            # boundary norm chain (merge->bc->recip->mul)
                            # resolves while f0..f2 stream; the f3 pair that
                            # needs the fresh attnT row comes after.
                            mid_hook()
                            mid_hook = None
                        for ec in range(2):
                            c0 = (ecg * 2 + ec) * 512
                            nc.tensor.matmul(
                                ops[ec], attnT[:, f, ss],
                                wo_sb[:, f, c0:c0 + 512],
                                start=(f == 0), stop=(f == QH - 1))
                    c0 = ecg * 1024
                    stg = o_stage.tile([P, 1024], F32, tag="stg",
                                       name="ostg")
                    nc.scalar.copy(out=stg[:, 0:512], in_=ops[0])
                    nc.sync.dma_start(out=out[ss, c0:c0 + 512],
                                      in_=stg[:, 0:512])
                    nc.vector.tensor_copy(out=stg[:, 512:1024], in_=ops[1])
                    nc.sync.dma_start(out=out[ss, c0 + 512:c0 + 1024],
                                      in_=stg[:, 512:1024])

            # attention q-window processing order: window 1 first (its
            # leading k-tiles are non-diagonal, so nothing waits on the DVE
            # queue right at the phase boundary), window 0 (all-diagonal,
            # cheapest) last.
            QWINS = [(0, 512), (1024, 512), (1536, 512), (512, 512)]

            with ExitStack() as attn_ctx:
                st_ps = attn_ctx.enter_context(
                    tc.tile_pool(name="stps", bufs=3, space="PSUM"))
                pv_ps = attn_ctx.enter_context(
                    tc.tile_pool(name="pvps", bufs=2, space="PSUM"))
                acc_pool = attn_ctx.enter_context(
                    tc.tile_pool(name="accp", bufs=6))

                def emit_norm(state):
                    # rowsum = all-partition reduce of the merged exp
                    # accumulator, broadcast in the same rank-128 matmul.
                    pv, accE, accO, h, qs, width = state
                    nc.vector.tensor_add(accE, accE, accO)
                    bc = bc_ps.tile([P, 512], F32, tag="bc", name="bcps")
                    nc.tensor.matmul(bc[:, 0:width], onesm_sb, accE,
                                     start=True, stop=True)
                    rec = norm_sc.tile([P, 512], F32, tag="rec", name="recb")
                    nc.vector.reciprocal_approx_fast(out=rec[:, 0:width],
                                                     in_=bc[:, 0:width])
                    nc.vector.tensor_mul(attnT[:, h, qs], pv,
                                         rec[:, 0:width])

                prev_norm = None
                oproj_q = []
                for wi, (q0, width) in enumerate(QWINS):
                    n_kt = (q0 + width) // P
                    jbase = q0 // P
                    qs = slice(q0, q0 + width)
                    for h in range(QH):
                        _mark(nc, f"attn_w{wi}_h{h}")
                        qr = roped[:, h, qs]
                        pv = pv_ps.tile([P, width], F32, tag="pv",
                                        name="pvps")
                        # rowsum accumulators: pairwise tree over the exp
                        # tiles, even k-tiles on DVE / odd on Pool — short
                        # dependency chains instead of a serial in-place sum.
                        accE = acc_pool.tile([P, width], F32R, tag="aE",
                                             name="accE")
                        accO = acc_pool.tile([P, width], F32R, tag="aO",
                                             name="accO")
                        # the first processed window has no o_proj filler:
                        # keep its whole rowsum chain on the (fast) DVE so
                        # nothing waits on Pool right after the phase switch.
                        odd_eng = nc.vector if wi == 0 else nc.gpsimd
                        nc.vector.memset(accE.bitcast(F32), 0.0)
                        odd_eng.memset(accO.bitcast(F32), 0.0)

                        def pv_sm(kt, pt, cs_, pv=pv, accE=accE, accO=accO,
                                  n_kt=n_kt, odd_eng=odd_eng):
                            nc.tensor.matmul(pv[:, cs_], v_nat[:, kt, :],
                                             pt[:, cs_], start=(kt == 0),
                                             stop=(kt == n_kt - 1))
                            if kt % 2 == 0:
                                nc.vector.tensor_add(accE[:, cs_],
                                                     accE[:, cs_],
                                                     pt[:, cs_])
                            else:
                                odd_eng.tensor_add(accO[:, cs_],
                                                   accO[:, cs_],
                                                   pt[:, cs_])

                        pend = None
                        for kt in range(n_kt):
                            j = kt - jbase
                            # diagonal blocks: columns < 128j are fully
                            # masked; never compute or read them.
                            c0 = max(j, 0) * P
                            cs_ = slice(c0, width)
                            stp = st_ps.tile([P, width], F32, tag="st",
                                             name="stps")
                            if j >= 0:
                                # causal mask = utri^T @ dmask_j, computed
                                # straight into the score PSUM bank by PE:
                                # masking costs no cross-engine hop at all.
                                nc.tensor.matmul(
                                    stp[:, cs_], utri_sb,
                                    dmask_sb[:, j, c0:width],
                                    start=True, stop=False)
                            nc.tensor.matmul(
                                stp[:, cs_], kt_row[:, kt * P:(kt + 1) * P],
                                qr[:, cs_], start=(j < 0), stop=True,
                                skip_group_check=True)
                            pt = pt_pool.tile([P, width], F32R, tag="pt",
                                              name="ptile")
                            nc.scalar.activation(out=pt[:, cs_],
                                                 in_=stp[:, cs_],
                                                 func=EXP, scale=SCALE)
                            if pend is not None:
                                pv_sm(*pend)
                            pend = (kt, pt, cs_)
                            if kt == 1:
                                hook = None
                                if prev_norm is not None:
                                    pn = prev_norm
                                    hook = (lambda pn=pn: emit_norm(pn))
                                    prev_norm = None
                                if oproj_q:
                                    if h == 0 and hook is not None:
                                        # window boundary: fold the norm of
                                        # the previous window's last head
                                        # into the o_proj block's ecg0.
                                        oproj(oproj_q.pop(0), o_ps,
                                              ["o0", "o1"], mid_hook=hook)
                                        hook = None
                                    else:
                                        oproj(oproj_q.pop(0), o_ps,
                                              ["o0", "o1"])
                                if hook is not None:
                                    hook()
                        pv_sm(*pend)
                        if wi == 0 and deferred_tp:
                            j, st_ = deferred_tp.pop(0)
                            tp = o_ps.tile([P, P], F32, tag="o0",
                                           name="tpps")
                            nc.tensor.transpose(
                                tp, qkv_last[:, QH + 1, j * P:(j + 1) * P],
                                id_sb)
                            nc.scalar.copy(out=v_nat[:, st_, :], in_=tp)
                        if wi == 1 and deferred_rope:
                            dq, dm, dsw = deferred_rope.pop(0)
                            rope_chunk(dq, dm, dsw, eng=nc.vector)
                            if h == QH - 1 and deferred_rope:
                                dq, dm, dsw = deferred_rope.pop(0)
                                rope_chunk(dq, dm, dsw, eng=nc.vector)
                        prev_norm = (pv, accE, accO, h, qs, width)
                    oproj_q.extend(range(q0 // P, (q0 + width) // P))
                _mark(nc, "drain")
                final_norm = prev_norm
            # attention PSUM pools released: the drain o_proj gets 4 banks
            # so eviction never blocks the next accumulation group.
            with tc.tile_pool(name="dps", bufs=1, space="PSUM") as drain_ps:
                for di, st in enumerate(oproj_q):
                    oproj(st, drain_ps, ["d0", "d1", "d2", "d3"],
                          mid_hook=(lambda: emit_norm(final_norm))
                          if di == 0 else None)


def _host_inputs(positions, hidden_states, w_qkv, w_o):
    """Build the 8 per-core input maps (host-side sharding + layout prep)."""
    x = np.ascontiguousarray(
        hidden_states.reshape(S, HID).T.astype(ml_dtypes.bfloat16))

    pos = positions.reshape(S).astype(np.float32)
    inv = (1.0 / (ROPE_THETA ** (np.arange(0, D, 2, dtype=np.float32) / D)))
    ang = inv[:, None] * pos[None, :]                      # [64, S]
    cos = np.concatenate([np.cos(ang), np.cos(ang)], 0).astype(np.float32)
    sinr = np.concatenate([np.sin(ang), -np.sin(ang)], 0).astype(np.float32)

    utri = np.triu(np.ones((P, P), np.float32), 1).astype(ml_dtypes.bfloat16)
    dmask = np.zeros((P, 4, 512), dtype=np.float32)
    for j in range(4):
        for r in range(P):
            c = r + P * j
            if c < 512:
                dmask[r, j, c] = MASK_NEG
    dmask = dmask.astype(ml_dtypes.bfloat16)

    ones_mat = np.ones((P, P), np.float32)
    ident = np.eye(P, dtype=np.float32)

    in_maps = []
    for i in range(NCORES):
        wq = w_qkv[:, i * FO:(i + 1) * FO]
        wk = w_qkv[:, N_HEADS * D + i * D: N_HEADS * D + (i + 1) * D]
        wv = w_qkv[:, (N_HEADS + N_KV) * D + i * D:
                   (N_HEADS + N_KV) * D + (i + 1) * D]
        wqkv_i = np.ascontiguousarray(
            np.concatenate([wq, wk, wv], axis=1).astype(ml_dtypes.bfloat16))
        wo_i = np.ascontiguousarray(
            w_o[i * FO:(i + 1) * FO, :].astype(ml_dtypes.bfloat16))
        in_maps.append({
            "xt": x, "wqkv": wqkv_i, "wo": wo_i, "cos": cos, "sinr": sinr,
            "utri": utri, "dmask": dmask, "ones_mat": ones_mat,
            "ident": ident,
        })
    return in_maps


def kernel(positions, hidden_states, w_qkv, w_o):
    positions = np.asarray(positions)
    hidden_states = np.asarray(hidden_states, dtype=np.float32)
    w_qkv = np.asarray(w_qkv, dtype=np.float32)
    w_o = np.asarray(w_o, dtype=np.float32)

    if "nc" not in _CACHE:
        _CACHE["nc"] = _build_nc()
    nc = _CACHE["nc"]

    in_maps = _host_inputs(positions, hidden_states, w_qkv, w_o)
    res = run_bass_kernel_spmd(nc, in_maps, list(range(NCORES)))

    acc = np.zeros((S, HID), dtype=np.float32)
    for c in range(NCORES):
        acc += res.results[c]["out"]
    return acc.reshape(1, S, HID)



# revision 10
# speedup vs baseline: 1.1043x; 1.1043x over previous
"""Mixtral-style GQA attention (B=1, S=2048, HID=4096, 32 q-heads / 8 kv-heads,
head_dim=128, NeoX RoPE, causal) on 8 Trainium2 NeuronCores.

Sharding: tensor-parallel over heads. Core i gets q-heads [4i..4i+3] and
kv-head i (w_qkv columns), plus the matching w_o rows. Each core computes a
full-shape partial of the output projection; the host sums the 8 partials
(the "all-reduce") and returns the full output.

Device layout notes:
 - All matmuls run in fp32r (TF32) at 1 cycle/row; every tensor consumed by an
   fp32r matmul is produced only by f32r-writing instructions (walrus checks).
 - hidden_states is passed pre-transposed (XT [HID, S]) so the QKV projection
   needs no on-device transpose: qkvT[f, s] = sum_h W[h, f] * XT[h, s].
 - ALL DMAs issue from the otherwise-idle SP queue: DMA sequencer config
   costs ~600ns each and serializes against whatever engine queue carries
   it (exp dispatch on ACT, PSUM-eviction copies at window boundaries).
 - Phase 1 keeps all 32 W k-tiles resident (the qkv scratch is per-window);
   window 0 interleaves each weight pair with its X pair on the single DMA
   queue so it computes at stream pace.
 - Attention: the causal mask is computed INTO the score PSUM bank by PE
   itself (utri^T @ dmask_j, with the score matmul accumulating on top), so
   masking costs no cross-engine hop. Scores run one k-tile ahead of PV to
   hide the exp latency; softmax rowsums accumulate on DVE (even k-tiles)
   and Pool (odd) in SBUF, reduced+broadcast by one rank-128 matmul per
   head; normalization for head h is emitted during head h+1 with the
   o_proj block of a completed window interleaved to keep PE fed.
 - Softmax skips max-subtraction (scores are O(10), exp stays finite in
   f32). The last phase-1 window's RoPE is deferred into the attention
   phase (it only feeds the qw3 window) so its ops never sit in front of
   early softmax work on the in-order DVE/Pool queues.
 - o_proj PSUM eviction alternates DVE/ACT to split the copy load.
"""
from contextlib import ExitStack

import ml_dtypes
import numpy as np

import concourse.bacc as bacc
import concourse.tile as tile
from concourse import mybir
from concourse.bass_utils import run_bass_kernel_spmd

# ---- problem constants (hardcoded per contest contract) ----
HID = 4096
S = 2048
N_HEADS = 32
N_KV = 8
D = 128                    # head_dim
NCORES = 8
QH = N_HEADS // NCORES     # 4 q-heads per core
FEAT = QH * D + 2 * D      # 768 per-core qkv output columns (q0..q3, k, v)
FO = QH * D                # 512 per-core attn features for o_proj
ROPE_THETA = 10000.0
SCALE = D ** -0.5
MASK_NEG = -30000.0

P = 128
BF16 = mybir.dt.bfloat16
F32 = mybir.dt.float32
F32R = mybir.dt.float32r
EXP = mybir.ActivationFunctionType.Exp

NKT = HID // P     # 32 hidden k-tiles
NSW = S // 512     # 4 seq windows
NM = FEAT // P     # 6 qkv m-tiles
NST = S // P       # 16 seq tiles

_CACHE = {}
_MARKERS = []   # (label, approx I-id) appended during _kernel for trace reading


def _mark(nc, label):
    _MARKERS.append((label, nc.next_id()))


def _build_nc():
    nc = bacc.Bacc("TRN2", target_bir_lowering=False, debug=False)

    xt = nc.dram_tensor("xt", [HID, S], BF16, kind="ExternalInput").ap()
    wqkv = nc.dram_tensor("wqkv", [HID, FEAT], BF16, kind="ExternalInput").ap()
    wo = nc.dram_tensor("wo", [FO, HID], BF16, kind="ExternalInput").ap()
    cos_d = nc.dram_tensor("cos", [D, S], F32, kind="ExternalInput").ap()
    sinr_d = nc.dram_tensor("sinr", [D, S], F32, kind="ExternalInput").ap()
    utri_d = nc.dram_tensor("utri", [P, P], BF16, kind="ExternalInput").ap()
    dmask_d = nc.dram_tensor("dmask", [P, 4, 512], BF16, kind="ExternalInput").ap()
    onesm_d = nc.dram_tensor("ones_mat", [P, P], F32R, kind="ExternalInput").ap()
    id_d = nc.dram_tensor("ident", [P, P], F32, kind="ExternalInput").ap()
    out = nc.dram_tensor("out", [S, HID], F32, kind="ExternalOutput").ap()

    with tile.TileContext(nc) as tc:
        _kernel(tc, xt, wqkv, wo, cos_d, sinr_d, utri_d, dmask_d, onesm_d, id_d, out)
    nc.compile()
    return nc


def _kernel(tc, xt, wqkv, wo, cos_d, sinr_d, utri_d, dmask_d, onesm_d, id_d, out):
    nc = tc.nc

    with ExitStack() as big:
        persist = big.enter_context(tc.tile_pool(name="persist", bufs=1))
        wo_pool = big.enter_context(tc.tile_pool(name="wop", bufs=1))
        wo3 = wo.rearrange("(ft p) e -> p ft e", p=P)
        wo_sb = wo_pool.tile([P, QH, HID], BF16)
        onesm_sb = persist.tile([P, P], F32R)
        id_sb = persist.tile([P, P], F32)
        utri_sb = persist.tile([P, P], BF16)
        dmask_sb = persist.tile([P, 4, 512], BF16)
        roped = persist.tile([P, QH + 1, S], F32R)  # rotated q0..q3, K
        v_nat = persist.tile([P, NST, D], F32R)     # V in [seq-tile, dim] blocks
        # cos/sin and the LAST window's qkv scratch + rope temps live in the
        # persistent pool: phase-3 SBUF pools must never wait on the final
        # rope chunks (they run on the slow Pool engine into the attention
        # phase), so nothing phase 3 allocates may overlap what they read.
        cos_sb = persist.tile([D, S], F32)
        sinr_sb = persist.tile([D, S], F32)
        qkv_last = persist.tile([P, NM, 512], F32)
        rt_last = [persist.tile([P, 512], F32, name=f"rtl{i}")
                   for i in range(2)]

        deferred_rope = []
        deferred_tp = []

        def rope_chunk(qkv, m, sw, eng, rtmp=None):
            """roped[:,m,win] = qkv*cos + rot_half(qkv)*sin over a window."""
            win = slice(sw * 512, (sw + 1) * 512)
            row = qkv[:, m, :]
            if rtmp is None:
                tmp = rt_last[m % 2]
            else:
                tmp = rtmp.tile([P, 512], F32, tag="rt", name="ropetmp")
            eng.tensor_mul(tmp[0:64, :], row[64:128, :],
                           sinr_sb[64:128, win])
            eng.tensor_mul(tmp[64:128, :], row[0:64, :],
                           sinr_sb[0:64, win])
            eng.tensor_mul(row, row, cos_sb[:, win])
            eng.tensor_add(roped[:, m, win], row, tmp)

        # ---- phase 1: qkvT = wqkv^T @ XT, rope chunks interleaved ----
        wq3 = wqkv.rearrange("(kt p) f -> p kt f", p=P)
        with tc.tile_pool(name="qkvf", bufs=1) as qkvf_pool, \
             tc.tile_pool(name="rtmp", bufs=3) as rtmp, \
             tc.tile_pool(name="wq", bufs=1) as wq_pool, \
             tc.tile_pool(name="xts", bufs=8) as xt_pool:

            w_pair = [wq_pool.tile([P, 2, FEAT], BF16, tag=f"w{k}",
                                   name=f"w{k}") for k in range(NKT // 2)]
            w_res = [w_pair[kt // 2][:, kt % 2, :] for kt in range(NKT)]

            with tc.tile_pool(name="qkps", bufs=1, space="PSUM") as qk_ps, \
                 tc.tile_pool(name="tps", bufs=2, space="PSUM") as tp_ps:
                for sw in range(NSW):
                    _mark(nc, f"phase1_sw{sw}")
                    ps = [qk_ps.tile([P, 512], F32, tag=f"m{m}",
                                     name=f"qkps{m}") for m in range(NM)]
                    xt2 = xt.rearrange("(kt p) s -> p kt s", p=P)
                    for kt in range(NKT):
                        if kt % 2 == 0:
                            if sw == 0:
                                # single-queue issue order: each weight pair
                                # right before the X pair that joins it, so
                                # window 0 computes at stream pace. The very
                                # first pairs stream in halves: subtile deps
                                # let kt0 start after just the first halves.
                                if kt == 0:
                                    nc.sync.dma_start(
                                        out=w_pair[0][:, 0, :],
                                        in_=wq3[:, 0, :])
                                else:
                                    nc.sync.dma_start(
                                        out=w_pair[kt // 2],
                                        in_=wq3[:, kt:kt + 2, :])
                            xpair = xt_pool.tile([P, 2, 512], BF16,
                                                 name="xts")
                            if sw == 0 and kt == 0:
                                nc.sync.dma_start(out=xpair[:, 0, :],
                                                  in_=xt2[:, 0, 0:512])
                                nc.sync.dma_start(out=w_pair[0][:, 1, :],
                                                  in_=wq3[:, 1, :])
                                nc.sync.dma_start(out=xpair[:, 1, :],
                                                  in_=xt2[:, 1, 0:512])
                            else:
                                nc.sync.dma_start(
                                    out=xpair,
                                    in_=xt2[:, kt:kt + 2,
                                            sw * 512:(sw + 1) * 512])
                        xts = xpair[:, kt % 2, :]
                        for m in range(NM):
                            nc.tensor.matmul(
                                ps[m], w_res[kt][:, m * P:(m + 1) * P], xts,
                                start=(kt == 0), stop=(kt == NKT - 1))
                    if sw == 0:
                        # window 0's rope only reads cos/sin[:, 0:512]:
                        # load just that chunk here; the rest rides the
                        # lighter windows so window-0's w+x stream keeps
                        # the full DMA bandwidth.
                        nc.sync.dma_start(out=id_sb, in_=id_d)
                        nc.sync.dma_start(out=cos_sb[:, 0:512],
                                          in_=cos_d[:, 0:512])
                        nc.sync.dma_start(out=sinr_sb[:, 0:512],
                                          in_=sinr_d[:, 0:512])
                    elif sw == 1:
                        nc.sync.dma_start(out=onesm_sb, in_=onesm_d)
                        nc.sync.dma_start(out=utri_sb, in_=utri_d)
                        for cch in (1, 2):
                            cw = slice(cch * 512, (cch + 1) * 512)
                            nc.sync.dma_start(out=cos_sb[:, cw],
                                              in_=cos_d[:, cw])
                            nc.sync.dma_start(out=sinr_sb[:, cw],
                                              in_=sinr_d[:, cw])
                    elif sw == 2:
                        nc.sync.dma_start(out=dmask_sb, in_=dmask_d)
                        nc.sync.dma_start(out=cos_sb[:, 1536:2048],
                                          in_=cos_d[:, 1536:2048])
                        nc.sync.dma_start(out=sinr_sb[:, 1536:2048],
                                          in_=sinr_d[:, 1536:2048])
                        # prefetch w_o (bf16) while the DMA queue is light
                        for c in range(8):
                            cw = slice(c * 512, (c + 1) * 512)
                            nc.sync.dma_start(out=wo_sb[:, :, cw],
                                              in_=wo3[:, :, cw])
                    if sw == NSW - 1:
                        qkv = qkv_last
                    else:
                        qkv = qkvf_pool.tile([P, NM, 512], F32, tag="qkv",
                                             name="qkvwin")
                    # eviction order frees the banks the next window touches
                    # first; alternating ACT/DVE halves the eviction chain.
                    # V (m=5) evicts first so the transposes start at once.
                    nc.scalar.copy(out=qkv[:, QH + 1, :], in_=ps[QH + 1])
                    nc.vector.tensor_copy(out=qkv[:, 0, :], in_=ps[0])
                    nc.scalar.copy(out=qkv[:, 1, :], in_=ps[1])
                    nc.vector.tensor_copy(out=qkv[:, 2, :], in_=ps[2])
                    nc.scalar.copy(out=qkv[:, 3, :], in_=ps[3])
                    nc.vector.tensor_copy(out=qkv[:, 4, :], in_=ps[4])
                    # V transpose for this window's 4 seq blocks; the
                    # last window's are deferred into phase 3 (their v_nat
                    # tiles are only read by the qw3 attention window) so
                    # the phase transition never waits on them.
                    for j in range(4):
                        st = 4 * sw + j
                        if sw == NSW - 1:
                            deferred_tp.append((j, st))
                            continue
                        tp = tp_ps.tile([P, P], F32, tag="tp", name="tpps")
                        nc.tensor.transpose(
                            tp, qkv[:, QH + 1, j * P:(j + 1) * P], id_sb)
                        nc.scalar.copy(out=v_nat[:, st, :], in_=tp)
                    if sw == NSW - 1:
                        # defer: the last window's rope is only needed by the
                        # qw3 attention window, and its Pool/DVE ops would
                        # sit in front of the first heads' softmax work on
                        # the in-order queues. Emitted inside phase 3.
                        deferred_rope.extend(
                            (qkv, m, sw) for m in range(QH + 1))
                    else:
                        for m in range(QH + 1):   # rope, DVE/Pool split
                            rope_chunk(qkv, m, sw,
                                       eng=(nc.gpsimd if m % 2 == 0
                                            else nc.vector), rtmp=rtmp)

        # ---- phase 3: attention with o_proj fully interleaved ----
        kt_row = roped[:, QH, :]
        with tc.tile_pool(name="atn", bufs=1) as atn_pool, \
             tc.tile_pool(name="pts", bufs=10) as pt_pool, \
             tc.tile_pool(name="nsc", bufs=2) as norm_sc, \
             tc.tile_pool(name="ost", bufs=6) as o_stage, \
             tc.tile_pool(name="ops", bufs=1, space="PSUM") as o_ps, \
             tc.tile_pool(name="bcps", bufs=1, space="PSUM") as bc_ps:
            attnT = atn_pool.tile([P, QH, S], BF16)

            def emit_ecg(st, ecg, pool, tags, mid_hook=None):
                """One 1024-col accumulation group of o_proj seq-tile st.

                Emitted one-at-a-time between attention k-tiles so each
                PSUM pair has several k-tile iterations to evict before its
                bank is reused (tags alternate pairs per ecg parity).
                """
                ss = slice(st * P, (st + 1) * P)
                ops = [pool.tile([P, 512], F32,
                                 tag=tags[(ecg * 2 + ec) % len(tags)],
                                 name="opsb") for ec in range(2)]
                for f in range(QH):
                    if f == QH - 1 and mid_hook is not None:
                        # boundary norm chain (merge->bc->recip->mul)
                        # resolves while f0..f2 stream; the f3 pair that
                        # needs the fresh attnT row comes after.
                        mid_hook()
                        mid_hook = None
                    for ec in range(2):
                        c0 = (ecg * 2 + ec) * 512
                        nc.tensor.matmul(
                            ops[ec], attnT[:, f, ss],
                            wo_sb[:, f, c0:c0 + 512],
                            start=(f == 0), stop=(f == QH - 1))
                c0 = ecg * 1024
                stg = o_stage.tile([P, 1024], F32, tag="stg", name="ostg")
                nc.scalar.copy(out=stg[:, 0:512], in_=ops[0])
                nc.sync.dma_start(out=out[ss, c0:c0 + 512],
                                  in_=stg[:, 0:512])
                nc.vector.tensor_copy(out=stg[:, 512:1024], in_=ops[1])
                nc.sync.dma_start(out=out[ss, c0 + 512:c0 + 1024],
                                  in_=stg[:, 512:1024])

            # attention q-window processing order: window 1 first (its
            # leading k-tiles are non-diagonal, so nothing waits on the DVE
            # queue right at the phase boundary), window 0 (all-diagonal,
            # cheapest) last.
            QWINS = [(0, 512), (1024, 512), (1536, 512), (512, 512)]

            with ExitStack() as attn_ctx:
                st_ps = attn_ctx.enter_context(
                    tc.tile_pool(name="stps", bufs=3, space="PSUM"))
                pv_ps = attn_ctx.enter_context(
                    tc.tile_pool(name="pvps", bufs=2, space="PSUM"))
                acc_pool = attn_ctx.enter_context(
                    tc.tile_pool(name="accp", bufs=6))
                OTAGS = ["o0", "o1"]

                def norm_merge(state):
                    # stage A, emitted at head end: merge the two rowsum
                    # accumulators on DVE so the bc matmul never has to wait
                    # for it when it reaches the PE queue one k-tile later.
                    pv, accE, accO, h, qs, width = state
                    nc.vector.tensor_add(accE, accE, accO)

                def norm_finish(state):
                    # stage B: rowsum = all-partition reduce of the merged
                    # accumulator, broadcast in the same rank-128 matmul.
                    pv, accE, accO, h, qs, width = state
                    bc = bc_ps.tile([P, 512], F32, tag="bc", name="bcps")
                    nc.tensor.matmul(bc[:, 0:width], onesm_sb, accE,
                                     start=True, stop=True)
                    rec = norm_sc.tile([P, 512], F32, tag="rec", name="recb")
                    nc.vector.reciprocal_approx_fast(out=rec[:, 0:width],
                                                     in_=bc[:, 0:width])
                    nc.vector.tensor_mul(attnT[:, h, qs], pv,
                                         rec[:, 0:width])

                prev_norm = None
                ecg_q = []
                for wi, (q0, width) in enumerate(QWINS):
                    n_kt = (q0 + width) // P
                    jbase = q0 // P
                    qs = slice(q0, q0 + width)
                    for h in range(QH):
                        _mark(nc, f"attn_w{wi}_h{h}")
                        qr = roped[:, h, qs]
                        pv = pv_ps.tile([P, width], F32, tag="pv",
                                        name="pvps")
                        # rowsum accumulators: pairwise tree over the exp
                        # tiles, split 2:1 DVE:Pool by k-tile (Pool streams
                        # ~2x slower per element) — short dependency chains
                        # instead of a serial in-place sum. First touch is a
                        # copy, so no memset is needed (the first k-tile of
                        # each group covers the full width, except in the
                        # all-diagonal window 0, which keeps accO's memset).
                        accE = acc_pool.tile([P, width], F32R, tag="aE",
                                             name="accE")
                        accO = acc_pool.tile([P, width], F32R, tag="aO",
                                             name="accO")
                        # the first processed window has no o_proj filler:
                        # keep its whole rowsum chain on the (fast) DVE so
                        # nothing waits on Pool right after the phase switch.
                        odd_eng = nc.vector if wi == 0 else nc.gpsimd
                        if wi == 0:
                            odd_eng.memset(accO.bitcast(F32), 0.0)
                        acc_first = {0: True, 1: wi != 0}

                        def pv_sm(kt, pt, cs_, pv=pv, accE=accE, accO=accO,
                                  n_kt=n_kt, odd_eng=odd_eng,
                                  acc_first=acc_first):
                            nc.tensor.matmul(pv[:, cs_], v_nat[:, kt, :],
                                             pt[:, cs_], start=(kt == 0),
                                             stop=(kt == n_kt - 1))
                            par = 0 if kt % 3 != 2 else 1
                            eng = nc.vector if par == 0 else odd_eng
                            acc = accE if par == 0 else accO
                            if acc_first[par]:
                                eng.tensor_copy(out=acc[:, cs_],
                                                in_=pt[:, cs_])
                                acc_first[par] = False
                            else:
                                eng.tensor_add(acc[:, cs_], acc[:, cs_],
                                               pt[:, cs_])

                        pend = None
                        for kt in range(n_kt):
                            j = kt - jbase
                            # diagonal blocks: columns < 128j are fully
                            # masked; never compute or read them.
                            c0 = max(j, 0) * P
                            cs_ = slice(c0, width)
                            stp = st_ps.tile([P, width], F32, tag="st",
                                             name="stps")
                            if j >= 0:
                                # causal mask = utri^T @ dmask_j, computed
                                # straight into the score PSUM bank by PE:
                                # masking costs no cross-engine hop at all.
                                nc.tensor.matmul(
                                    stp[:, cs_], utri_sb,
                                    dmask_sb[:, j, c0:width],
                                    start=True, stop=False)
                            nc.tensor.matmul(
                                stp[:, cs_], kt_row[:, kt * P:(kt + 1) * P],
                                qr[:, cs_], start=(j < 0), stop=True,
                                skip_group_check=True)
                            pt = pt_pool.tile([P, width], F32R, tag="pt",
                                              name="ptile")
                            nc.scalar.activation(out=pt[:, cs_],
                                                 in_=stp[:, cs_],
                                                 func=EXP, scale=SCALE)
                            if pend is not None:
                                pv_sm(*pend)
                            pend = (kt, pt, cs_)
                            if kt in (1, 3, 5, 7):
                                hook = None
                                if kt == 1 and prev_norm is not None:
                                    pn = prev_norm
                                    hook = (lambda pn=pn: norm_finish(pn))
                                    prev_norm = None
                                if ecg_q:
                                    est, eg = ecg_q.pop(0)
                                    if h == 0 and kt == 1 and hook is not None:
                                        # window boundary: fold the norm of
                                        # the previous window's last head
                                        # into this ecg (its f3 reads the
                                        # freshly normalized attnT row).
                                        emit_ecg(est, eg, o_ps, OTAGS,
                                                 mid_hook=hook)
                                    else:
                                        # mid-window: norm first, so its DVE
                                        # ops sit ahead of the eviction
                                        # copies in the in-order queues.
                                        if hook is not None:
                                            hook()
                                        emit_ecg(est, eg, o_ps, OTAGS)
                                elif hook is not None:
                                    hook()
                        pv_sm(*pend)
                        if wi == 0 and deferred_tp:
                            j, st_ = deferred_tp.pop(0)
                            # same shape as the o_proj tiles so tag o0 keeps
                            # a single uniform buffer; only [:, :P] is used.
                            tp = o_ps.tile([P, 512], F32, tag="o0",
                                           name="tpps")
                            nc.tensor.transpose(
                                tp[:, 0:P],
                                qkv_last[:, QH + 1, j * P:(j + 1) * P],
                                id_sb)
                            nc.scalar.copy(out=v_nat[:, st_, :],
                                           in_=tp[:, 0:P])
                        if wi == 1 and deferred_rope:
                            # Pool has slack here; DVE carries the norm
                            # chain + evictions.
                            dq, dm, dsw = deferred_rope.pop(0)
                            rope_chunk(dq, dm, dsw, eng=nc.gpsimd)
                            if h == QH - 1 and deferred_rope:
                                dq, dm, dsw = deferred_rope.pop(0)
                                rope_chunk(dq, dm, dsw, eng=nc.gpsimd)
                        prev_norm = (pv, accE, accO, h, qs, width)
                        norm_merge(prev_norm)
                    ecg_q.extend((st, g)
                                 for st in range(q0 // P, (q0 + width) // P)
                                 for g in range(4))
                _mark(nc, "drain")
                final_norm = prev_norm
            # attention PSUM pools released: the drain o_proj gets 4 banks
            # so eviction never blocks the next accumulation group.
            with tc.tile_pool(name="dps", bufs=1, space="PSUM") as drain_ps:
                for di, (st, g) in enumerate(ecg_q):
                    emit_ecg(st, g, drain_ps, ["d0", "d1", "d2", "d3"],
                             mid_hook=(lambda: norm_finish(final_norm))
                             if di == 0 else None)


def _host_inputs(positions, hidden_states, w_qkv, w_o):
    """Build the 8 per-core input maps (host-side sharding + layout prep)."""
    x = np.ascontiguousarray(
        hidden_states.reshape(S, HID).T.astype(ml_dtypes.bfloat16))

    pos = positions.reshape(S).astype(np.float32)
    inv = (1.0 / (ROPE_THETA ** (np.arange(0, D, 2, dtype=np.float32) / D)))
    ang = inv[:, None] * pos[None, :]                      # [64, S]
    cos = np.concatenate([np.cos(ang), np.cos(ang)], 0).astype(np.float32)
    sinr = np.concatenate([np.sin(ang), -np.sin(ang)], 0).astype(np.float32)

    utri = np.triu(np.ones((P, P), np.float32), 1).astype(ml_dtypes.bfloat16)
    dmask = np.zeros((P, 4, 512), dtype=np.float32)
    for j in range(4):
        for r in range(P):
            c = r + P * j
            if c < 512:
                dmask[r, j, c] = MASK_NEG
    dmask = dmask.astype(ml_dtypes.bfloat16)

    ones_mat = np.ones((P, P), np.float32)
    ident = np.eye(P, dtype=np.float32)

    in_maps = []
    for i in range(NCORES):
        wq = w_qkv[:, i * FO:(i + 1) * FO]
        wk = w_qkv[:, N_HEADS * D + i * D: N_HEADS * D + (i + 1) * D]
        wv = w_qkv[:, (N_HEADS + N_KV) * D + i * D:
                   (N_HEADS + N_KV) * D + (i + 1) * D]
        wqkv_i = np.ascontiguousarray(
            np.concatenate([wq, wk, wv], axis=1).astype(ml_dtypes.bfloat16))
        wo_i = np.ascontiguousarray(
            w_o[i * FO:(i + 1) * FO, :].astype(ml_dtypes.bfloat16))
        in_maps.append({
            "xt": x, "wqkv": wqkv_i, "wo": wo_i, "cos": cos, "sinr": sinr,
            "utri": utri, "dmask": dmask, "ones_mat": ones_mat,
            "ident": ident,
        })
    return in_maps


def kernel(positions, hidden_states, w_qkv, w_o):
    positions = np.asarray(positions)
    hidden_states = np.asarray(hidden_states, dtype=np.float32)
    w_qkv = np.asarray(w_qkv, dtype=np.float32)
    w_o = np.asarray(w_o, dtype=np.float32)

    if "nc" not in _CACHE:
        _CACHE["nc"] = _build_nc()
    nc = _CACHE["nc"]

    in_maps = _host_inputs(positions, hidden_states, w_qkv, w_o)
    res = run_bass_kernel_spmd(nc, in_maps, list(range(NCORES)))

    acc = np.zeros((S, HID), dtype=np.float32)
    for c in range(NCORES):
        acc += res.results[c]["out"]
    return acc.reshape(1, S, HID)



# revision 14
# speedup vs baseline: 1.2557x; 1.1371x over previous
"""Mixtral-style GQA attention (B=1, S=2048, HID=4096, 32 q-heads / 8 kv-heads,
head_dim=128, NeoX RoPE, causal) on 8 Trainium2 NeuronCores.

Sharding: tensor-parallel over heads. Core i gets q-heads [4i..4i+3] and
kv-head i (w_qkv columns), plus the matching w_o rows. Each core computes a
full-shape partial of the output projection; the host sums the 8 partials
(the "all-reduce") and returns the full output.

Device layout notes:
 - All matmuls run in fp32r (TF32) at 1 cycle/row; every tensor consumed by an
   fp32r matmul is produced only by f32r-writing instructions (walrus checks).
 - hidden_states is passed pre-transposed (XT [HID, S]) so the QKV projection
   needs no on-device transpose: qkvT[f, s] = sum_h W[h, f] * XT[h, s].
 - ALL DMAs issue from the otherwise-idle SP queue: DMA sequencer config
   costs ~600ns each and serializes against whatever engine queue carries
   it (exp dispatch on ACT, PSUM-eviction copies at window boundaries).
 - Phase 1 keeps all 32 W k-tiles resident (the qkv scratch is per-window);
   window 0 interleaves each weight pair with its X pair on the single DMA
   queue so it computes at stream pace.
 - Attention: the causal mask is computed INTO the score PSUM bank by PE
   itself (utri^T @ dmask_j, with the score matmul accumulating on top), so
   masking costs no cross-engine hop. Scores run one k-tile ahead of PV to
   hide the exp latency; softmax rowsums accumulate on DVE (even k-tiles)
   and Pool (odd) in SBUF, reduced+broadcast by one rank-128 matmul per
   head; normalization for head h is emitted during head h+1 with the
   o_proj block of a completed window interleaved to keep PE fed.
 - Softmax skips max-subtraction (scores are O(10), exp stays finite in
   f32). The last phase-1 window's RoPE is deferred into the attention
   phase (it only feeds the qw3 window) so its ops never sit in front of
   early softmax work on the in-order DVE/Pool queues.
 - o_proj PSUM eviction alternates DVE/ACT to split the copy load.
"""
from contextlib import ExitStack

import ml_dtypes
import numpy as np

import concourse.bacc as bacc
import concourse.tile as tile
from concourse import mybir
from concourse.bass_utils import run_bass_kernel_spmd

# ---- problem constants (hardcoded per contest contract) ----
HID = 4096
S = 2048
N_HEADS = 32
N_KV = 8
D = 128                    # head_dim
NCORES = 8
QH = N_HEADS // NCORES     # 4 q-heads per core
FEAT = QH * D + 2 * D      # 768 per-core qkv output columns (q0..q3, k, v)
FO = QH * D                # 512 per-core attn features for o_proj
ROPE_THETA = 10000.0
SCALE = D ** -0.5
MASK_NEG = -30000.0

P = 128
BF16 = mybir.dt.bfloat16
F32 = mybir.dt.float32
F32R = mybir.dt.float32r
EXP = mybir.ActivationFunctionType.Exp

NKT = HID // P     # 32 hidden k-tiles
NSW = S // 512     # 4 seq windows
NM = FEAT // P     # 6 qkv m-tiles
NST = S // P       # 16 seq tiles

_CACHE = {}
_MARKERS = []   # (label, approx I-id) appended during _kernel for trace reading


def _mark(nc, label):
    _MARKERS.append((label, nc.next_id()))


def _build_nc():
    nc = bacc.Bacc("TRN2", target_bir_lowering=False, debug=False)

    xt = nc.dram_tensor("xt", [HID, S], BF16, kind="ExternalInput").ap()
    wqkv = nc.dram_tensor("wqkv", [HID, FEAT], BF16, kind="ExternalInput").ap()
    wo = nc.dram_tensor("wo", [FO, HID], BF16, kind="ExternalInput").ap()
    cos_d = nc.dram_tensor("cos", [D, S], F32, kind="ExternalInput").ap()
    sinr_d = nc.dram_tensor("sinr", [D, S], F32, kind="ExternalInput").ap()
    utri_d = nc.dram_tensor("utri", [P, P], BF16, kind="ExternalInput").ap()
    dmask_d = nc.dram_tensor("dmask", [P, 4, 512], BF16, kind="ExternalInput").ap()
    onesm_d = nc.dram_tensor("ones_mat", [P, P], F32R, kind="ExternalInput").ap()
    id_d = nc.dram_tensor("ident", [P, P], F32, kind="ExternalInput").ap()
    out = nc.dram_tensor("out", [S, HID], F32, kind="ExternalOutput").ap()

    with tile.TileContext(nc) as tc:
        _kernel(tc, xt, wqkv, wo, cos_d, sinr_d, utri_d, dmask_d, onesm_d, id_d, out)
    nc.compile()
    return nc


def _kernel(tc, xt, wqkv, wo, cos_d, sinr_d, utri_d, dmask_d, onesm_d, id_d, out):
    nc = tc.nc

    with ExitStack() as big:
        persist = big.enter_context(tc.tile_pool(name="persist", bufs=1))
        wo_pool = big.enter_context(tc.tile_pool(name="wop", bufs=1))
        wo3 = wo.rearrange("(ft p) e -> p ft e", p=P)
        wo_sb = wo_pool.tile([P, QH, HID], BF16)
        onesm_sb = persist.tile([P, P], F32R)
        id_sb = persist.tile([P, P], F32)
        utri_sb = persist.tile([P, P], BF16)
        dmask_sb = persist.tile([P, 4, 512], BF16)
        roped = persist.tile([P, QH + 1, S], F32R)  # rotated q0..q3, K
        v_nat = persist.tile([P, NST, D], F32R)     # V in [seq-tile, dim] blocks
        # cos/sin and the LAST window's qkv scratch + rope temps live in the
        # persistent pool: phase-3 SBUF pools must never wait on the final
        # rope chunks (they run on the slow Pool engine into the attention
        # phase), so nothing phase 3 allocates may overlap what they read.
        cos_sb = persist.tile([D, S], F32)
        sinr_sb = persist.tile([D, S], F32)
        qkv_last = persist.tile([P, NM, 512], F32)
        rt_last = [persist.tile([P, 512], F32, name=f"rtl{i}")
                   for i in range(2)]

        deferred_rope = []
        deferred_tp = []

        def rope_chunk(qkv, m, sw, eng, rtmp=None):
            """roped[:,m,win] = qkv*cos + rot_half(qkv)*sin over a window."""
            win = slice(sw * 512, (sw + 1) * 512)
            row = qkv[:, m, :]
            if rtmp is None:
                tmp = rt_last[m % 2]
            else:
                tmp = rtmp.tile([P, 512], F32, tag="rt", name="ropetmp")
            eng.tensor_mul(tmp[0:64, :], row[64:128, :],
                           sinr_sb[64:128, win])
            eng.tensor_mul(tmp[64:128, :], row[0:64, :],
                           sinr_sb[0:64, win])
            eng.tensor_mul(row, row, cos_sb[:, win])
            eng.tensor_add(roped[:, m, win], row, tmp)

        # ---- phase 1: qkvT = wqkv^T @ XT, rope chunks interleaved ----
        wq3 = wqkv.rearrange("(kt p) f -> p kt f", p=P)
        with tc.tile_pool(name="qkvf", bufs=1) as qkvf_pool, \
             tc.tile_pool(name="rtmp", bufs=3) as rtmp, \
             tc.tile_pool(name="wq", bufs=1) as wq_pool, \
             tc.tile_pool(name="xts", bufs=8) as xt_pool:

            w_pair = [wq_pool.tile([P, 2, FEAT], BF16, tag=f"w{k}",
                                   name=f"w{k}") for k in range(NKT // 2)]
            w_res = [w_pair[kt // 2][:, kt % 2, :] for kt in range(NKT)]

            with tc.tile_pool(name="qkps", bufs=1, space="PSUM") as qk_ps, \
                 tc.tile_pool(name="tps", bufs=2, space="PSUM") as tp_ps:
                for sw in range(NSW):
                    _mark(nc, f"phase1_sw{sw}")
                    ps = [qk_ps.tile([P, 512], F32, tag=f"m{m}",
                                     name=f"qkps{m}") for m in range(NM)]
                    xt2 = xt.rearrange("(kt p) s -> p kt s", p=P)
                    for kt in range(NKT):
                        if kt % 2 == 0:
                            if sw == 0:
                                # single-queue issue order: each weight pair
                                # right before the X pair that joins it, so
                                # window 0 computes at stream pace. The very
                                # first pairs stream in halves: subtile deps
                                # let kt0 start after just the first halves.
                                if kt == 0:
                                    nc.sync.dma_start(
                                        out=w_pair[0][:, 0, :],
                                        in_=wq3[:, 0, :])
                                else:
                                    nc.sync.dma_start(
                                        out=w_pair[kt // 2],
                                        in_=wq3[:, kt:kt + 2, :])
                            xpair = xt_pool.tile([P, 2, 512], BF16,
                                                 name="xts")
                            if sw == 0 and kt == 0:
                                nc.sync.dma_start(out=xpair[:, 0, :],
                                                  in_=xt2[:, 0, 0:512])
                                nc.sync.dma_start(out=w_pair[0][:, 1, :],
                                                  in_=wq3[:, 1, :])
                                nc.sync.dma_start(out=xpair[:, 1, :],
                                                  in_=xt2[:, 1, 0:512])
                            else:
                                nc.sync.dma_start(
                                    out=xpair,
                                    in_=xt2[:, kt:kt + 2,
                                            sw * 512:(sw + 1) * 512])
                        xts = xpair[:, kt % 2, :]
                        for m in range(NM):
                            nc.tensor.matmul(
                                ps[m], w_res[kt][:, m * P:(m + 1) * P], xts,
                                start=(kt == 0), stop=(kt == NKT - 1))
                    if sw == 0:
                        # window 0's rope only reads cos/sin[:, 0:512]:
                        # load just that chunk here; the rest rides the
                        # lighter windows so window-0's w+x stream keeps
                        # the full DMA bandwidth.
                        nc.sync.dma_start(out=id_sb, in_=id_d)
                        nc.sync.dma_start(out=cos_sb[:, 0:512],
                                          in_=cos_d[:, 0:512])
                        nc.sync.dma_start(out=sinr_sb[:, 0:512],
                                          in_=sinr_d[:, 0:512])
                    elif sw == 1:
                        nc.sync.dma_start(out=onesm_sb, in_=onesm_d)
                        nc.sync.dma_start(out=utri_sb, in_=utri_d)
                        for cch in (1, 2):
                            cw = slice(cch * 512, (cch + 1) * 512)
                            nc.sync.dma_start(out=cos_sb[:, cw],
                                              in_=cos_d[:, cw])
                            nc.sync.dma_start(out=sinr_sb[:, cw],
                                              in_=sinr_d[:, cw])
                    elif sw == 2:
                        nc.sync.dma_start(out=dmask_sb, in_=dmask_d)
                        nc.sync.dma_start(out=cos_sb[:, 1536:2048],
                                          in_=cos_d[:, 1536:2048])
                        nc.sync.dma_start(out=sinr_sb[:, 1536:2048],
                                          in_=sinr_d[:, 1536:2048])
                        # prefetch w_o (bf16) while the DMA queue is light
                        for c in range(8):
                            cw = slice(c * 512, (c + 1) * 512)
                            nc.sync.dma_start(out=wo_sb[:, :, cw],
                                              in_=wo3[:, :, cw])
                    if sw == NSW - 1:
                        qkv = qkv_last
                    else:
                        qkv = qkvf_pool.tile([P, NM, 512], F32, tag="qkv",
                                             name="qkvwin")
                    # eviction order frees the banks the next window touches
                    # first; alternating ACT/DVE halves the eviction chain.
                    # V (m=5) evicts first so the transposes start at once.
                    nc.scalar.copy(out=qkv[:, QH + 1, :], in_=ps[QH + 1])
                    nc.vector.tensor_copy(out=qkv[:, 0, :], in_=ps[0])
                    nc.scalar.copy(out=qkv[:, 1, :], in_=ps[1])
                    nc.vector.tensor_copy(out=qkv[:, 2, :], in_=ps[2])
                    nc.scalar.copy(out=qkv[:, 3, :], in_=ps[3])
                    nc.vector.tensor_copy(out=qkv[:, 4, :], in_=ps[4])
                    # V transpose for this window's 4 seq blocks; the
                    # last window's are deferred into phase 3 (their v_nat
                    # tiles are only read by the qw3 attention window) so
                    # the phase transition never waits on them.
                    for j in range(4):
                        st = 4 * sw + j
                        if sw == NSW - 1:
                            deferred_tp.append((j, st))
                            continue
                        tp = tp_ps.tile([P, P], F32, tag="tp", name="tpps")
                        nc.tensor.transpose(
                            tp, qkv[:, QH + 1, j * P:(j + 1) * P], id_sb)
                        nc.scalar.copy(out=v_nat[:, st, :], in_=tp)
                    if sw == NSW - 1:
                        # defer: the last window's rope is only needed by the
                        # qw3 attention window, and its Pool/DVE ops would
                        # sit in front of the first heads' softmax work on
                        # the in-order queues. Emitted inside phase 3.
                        deferred_rope.extend(
                            (qkv, m, sw) for m in range(QH + 1))
                    else:
                        for m in range(QH + 1):   # rope, DVE/Pool split
                            rope_chunk(qkv, m, sw,
                                       eng=(nc.gpsimd if m % 2 == 0
                                            else nc.vector), rtmp=rtmp)

        # ---- phase 3: attention with o_proj fully interleaved ----
        kt_row = roped[:, QH, :]
        with tc.tile_pool(name="atn", bufs=1) as atn_pool, \
             tc.tile_pool(name="pts", bufs=10) as pt_pool, \
             tc.tile_pool(name="nsc", bufs=2) as norm_sc, \
             tc.tile_pool(name="ost", bufs=6) as o_stage, \
             tc.tile_pool(name="ops", bufs=1, space="PSUM") as o_ps, \
             tc.tile_pool(name="bcps", bufs=1, space="PSUM") as bc_ps:
            attnT = atn_pool.tile([P, QH, S], BF16)

            def emit_ecg(st, ecg, pool, tags, mid_hook=None):
                """One 1024-col accumulation group of o_proj seq-tile st.

                Emitted one-at-a-time between attention k-tiles so each
                PSUM pair has several k-tile iterations to evict before its
                bank is reused (tags alternate pairs per ecg parity).
                """
                ss = slice(st * P, (st + 1) * P)
                ops = [pool.tile([P, 512], F32,
                                 tag=tags[(ecg * 2 + ec) % len(tags)],
                                 name="opsb") for ec in range(2)]
                for f in range(QH):
                    if f == QH - 1 and mid_hook is not None:
                        # boundary norm chain (merge->bc->recip->mul)
                        # resolves while f0..f2 stream; the f3 pair that
                        # needs the fresh attnT row comes after.
                        mid_hook()
                        mid_hook = None
                    for ec in range(2):
                        c0 = (ecg * 2 + ec) * 512
                        nc.tensor.matmul(
                            ops[ec], attnT[:, f, ss],
                            wo_sb[:, f, c0:c0 + 512],
                            start=(f == 0), stop=(f == QH - 1))
                c0 = ecg * 1024
                stg = o_stage.tile([P, 1024], F32, tag="stg", name="ostg")
                nc.scalar.copy(out=stg[:, 0:512], in_=ops[0])
                nc.sync.dma_start(out=out[ss, c0:c0 + 512],
                                  in_=stg[:, 0:512])
                nc.vector.tensor_copy(out=stg[:, 512:1024], in_=ops[1])
                nc.sync.dma_start(out=out[ss, c0 + 512:c0 + 1024],
                                  in_=stg[:, 512:1024])

            # attention q-window processing order: window 1 first (its
            # leading k-tiles are non-diagonal, so nothing waits on the DVE
            # queue right at the phase boundary), window 0 (all-diagonal,
            # cheapest) last.
            QWINS = [(0, 512), (1024, 512), (1536, 512), (512, 512)]

            with ExitStack() as attn_ctx:
                st_ps = attn_ctx.enter_context(
                    tc.tile_pool(name="stps", bufs=3, space="PSUM"))
                pv_ps = attn_ctx.enter_context(
                    tc.tile_pool(name="pvps", bufs=2, space="PSUM"))
                acc_pool = attn_ctx.enter_context(
                    tc.tile_pool(name="accp", bufs=6))
                OTAGS = ["o0", "o1"]

                def norm_merge(state, eng):
                    # stage A, emitted at head end: merge the two rowsum
                    # accumulators so the bc matmul never has to wait for it
                    # when it reaches the PE queue one k-tile later.
                    pv, accE, accO, h, qs, width = state
                    eng.tensor_add(accE, accE, accO)

                def norm_finish(state):
                    # stage B: rowsum = all-partition reduce of the merged
                    # accumulator, broadcast in the same rank-128 matmul.
                    pv, accE, accO, h, qs, width = state
                    bc = bc_ps.tile([P, 512], F32, tag="bc", name="bcps")
                    nc.tensor.matmul(bc[:, 0:width], onesm_sb, accE,
                                     start=True, stop=True)
                    rec = norm_sc.tile([P, 512], F32, tag="rec", name="recb")
                    nc.vector.reciprocal_approx_fast(out=rec[:, 0:width],
                                                     in_=bc[:, 0:width])
                    nc.vector.tensor_mul(attnT[:, h, qs], pv,
                                         rec[:, 0:width])

                prev_norm = None
                ecg_q = []
                for wi, (q0, width) in enumerate(QWINS):
                    n_kt = (q0 + width) // P
                    jbase = q0 // P
                    qs = slice(q0, q0 + width)
                    for h in range(QH):
                        _mark(nc, f"attn_w{wi}_h{h}")
                        qr = roped[:, h, qs]
                        pv = pv_ps.tile([P, width], F32, tag="pv",
                                        name="pvps")
                        # rowsum accumulators: pairwise tree over the exp
                        # tiles, split DVE:Pool by k-tile (Pool streams ~2x
                        # slower per element, so it gets ~1/3; the final
                        # k-tile always lands on DVE so the merge never waits
                        # cross-engine on a late Pool add). First touch is a
                        # copy, so no memset is needed when it covers the
                        # full width; window 0's accO (first touch on a
                        # diagonal tile) keeps its memset.
                        accE = acc_pool.tile([P, width], F32R, tag="aE",
                                             name="accE")
                        accO = acc_pool.tile([P, width], F32R, tag="aO",
                                             name="accO")
                        if wi == 0:
                            nc.gpsimd.memset(accO.bitcast(F32), 0.0)
                        acc_first = {0: True, 1: wi != 0}

                        def pool_par(kt, n_kt=n_kt, wi=wi):
                            if kt == n_kt - 1:
                                return 0
                            if wi == 0:
                                return kt % 2
                            return 1 if kt % 3 == 1 else 0

                        def pv_sm(kt, pt, cs_, pv=pv, accE=accE, accO=accO,
                                  n_kt=n_kt, acc_first=acc_first,
                                  pool_par=pool_par):
                            nc.tensor.matmul(pv[:, cs_], v_nat[:, kt, :],
                                             pt[:, cs_], start=(kt == 0),
                                             stop=(kt == n_kt - 1))
                            par = pool_par(kt)
                            eng = nc.vector if par == 0 else nc.gpsimd
                            acc = accE if par == 0 else accO
                            if acc_first[par]:
                                eng.tensor_copy(out=acc[:, cs_],
                                                in_=pt[:, cs_])
                                acc_first[par] = False
                            else:
                                eng.tensor_add(acc[:, cs_], acc[:, cs_],
                                               pt[:, cs_])

                        pend = None
                        for kt in range(n_kt):
                            j = kt - jbase
                            # diagonal blocks: columns < 128j are fully
                            # masked; never compute or read them.
                            c0 = max(j, 0) * P
                            cs_ = slice(c0, width)
                            stp = st_ps.tile([P, width], F32, tag="st",
                                             name="stps")
                            if j >= 0:
                                # causal mask = utri^T @ dmask_j, computed
                                # straight into the score PSUM bank by PE:
                                # masking costs no cross-engine hop at all.
                                nc.tensor.matmul(
                                    stp[:, cs_], utri_sb,
                                    dmask_sb[:, j, c0:width],
                                    start=True, stop=False)
                            nc.tensor.matmul(
                                stp[:, cs_], kt_row[:, kt * P:(kt + 1) * P],
                                qr[:, cs_], start=(j < 0), stop=True,
                                skip_group_check=True)
                            pt = pt_pool.tile([P, width], F32R, tag="pt",
                                              name="ptile")
                            nc.scalar.activation(out=pt[:, cs_],
                                                 in_=stp[:, cs_],
                                                 func=EXP, scale=SCALE)
                            if pend is not None:
                                pv_sm(*pend)
                            pend = (kt, pt, cs_)
                            if kt in (1, 3, 5, 7):
                                hook = None
                                if kt == 1 and prev_norm is not None:
                                    pn = prev_norm
                                    hook = (lambda pn=pn: norm_finish(pn))
                                    prev_norm = None
                                if ecg_q:
                                    est, eg = ecg_q.pop(0)
                                    if h == 0 and kt == 1 and hook is not None:
                                        # window boundary: fold the norm of
                                        # the previous window's last head
                                        # into this ecg (its f3 reads the
                                        # freshly normalized attnT row).
                                        emit_ecg(est, eg, o_ps, OTAGS,
                                                 mid_hook=hook)
                                    else:
                                        # mid-window: ecg first — its 8
                                        # matmuls cover the bc matmul's wait
                                        # for the rowsum merge on DVE.
                                        emit_ecg(est, eg, o_ps, OTAGS)
                                        if hook is not None:
                                            hook()
                                elif hook is not None:
                                    hook()
                        pv_sm(*pend)
                        if wi == 0 and deferred_tp:
                            j, st_ = deferred_tp.pop(0)
                            # same shape as the o_proj tiles so tag o0 keeps
                            # a single uniform buffer; only [:, :P] is used.
                            tp = o_ps.tile([P, 512], F32, tag="o0",
                                           name="tpps")
                            nc.tensor.transpose(
                                tp[:, 0:P],
                                qkv_last[:, QH + 1, j * P:(j + 1) * P],
                                id_sb)
                            nc.scalar.copy(out=v_nat[:, st_, :],
                                           in_=tp[:, 0:P])
                        if wi == 1 and deferred_rope:
                            # Pool has slack here; DVE carries the norm
                            # chain + evictions.
                            dq, dm, dsw = deferred_rope.pop(0)
                            rope_chunk(dq, dm, dsw, eng=nc.gpsimd)
                            if h == QH - 1 and deferred_rope:
                                dq, dm, dsw = deferred_rope.pop(0)
                                rope_chunk(dq, dm, dsw, eng=nc.gpsimd)
                        prev_norm = (pv, accE, accO, h, qs, width)
                        norm_merge(prev_norm,
                                   nc.gpsimd if wi == 0 else nc.vector)
                    ecg_q.extend((st, g)
                                 for st in range(q0 // P, (q0 + width) // P)
                                 for g in range(4))
                _mark(nc, "drain")
                final_norm = prev_norm
            # attention PSUM pools released: the drain o_proj gets 4 banks
            # so eviction never blocks the next accumulation group.
            with tc.tile_pool(name="dps", bufs=1, space="PSUM") as drain_ps:
                for di, (st, g) in enumerate(ecg_q):
                    emit_ecg(st, g, drain_ps, ["d0", "d1", "d2", "d3"],
                             mid_hook=(lambda: norm_finish(final_norm))
                             if di == 0 else None)


def _host_inputs(positions, hidden_states, w_qkv, w_o):
    """Build the 8 per-core input maps (host-side sharding + layout prep)."""
    x = np.ascontiguousarray(
        hidden_states.reshape(S, HID).T.astype(ml_dtypes.bfloat16))

    pos = positions.reshape(S).astype(np.float32)
    inv = (1.0 / (ROPE_THETA ** (np.arange(0, D, 2, dtype=np.float32) / D)))
    ang = inv[:, None] * pos[None, :]                      # [64, S]
    cos = np.concatenate([np.cos(ang), np.cos(ang)], 0).astype(np.float32)
    sinr = np.concatenate([np.sin(ang), -np.sin(ang)], 0).astype(np.float32)

    utri = np.triu(np.ones((P, P), np.float32), 1).astype(ml_dtypes.bfloat16)
    dmask = np.zeros((P, 4, 512), dtype=np.float32)
    for j in range(4):
        for r in range(P):
            c = r + P * j
            if c < 512:
                dmask[r, j, c] = MASK_NEG
    dmask = dmask.astype(ml_dtypes.bfloat16)

    ones_mat = np.ones((P, P), np.float32)
    ident = np.eye(P, dtype=np.float32)

    in_maps = []
    for i in range(NCORES):
        wq = w_qkv[:, i * FO:(i + 1) * FO]
        wk = w_qkv[:, N_HEADS * D + i * D: N_HEADS * D + (i + 1) * D]
        wv = w_qkv[:, (N_HEADS + N_KV) * D + i * D:
                   (N_HEADS + N_KV) * D + (i + 1) * D]
        wqkv_i = np.ascontiguousarray(
            np.concatenate([wq, wk, wv], axis=1).astype(ml_dtypes.bfloat16))
        wo_i = np.ascontiguousarray(
            w_o[i * FO:(i + 1) * FO, :].astype(ml_dtypes.bfloat16))
        in_maps.append({
            "xt": x, "wqkv": wqkv_i, "wo": wo_i, "cos": cos, "sinr": sinr,
            "utri": utri, "dmask": dmask, "ones_mat": ones_mat,
            "ident": ident,
        })
    return in_maps


def kernel(positions, hidden_states, w_qkv, w_o):
    positions = np.asarray(positions)
    hidden_states = np.asarray(hidden_states, dtype=np.float32)
    w_qkv = np.asarray(w_qkv, dtype=np.float32)
    w_o = np.asarray(w_o, dtype=np.float32)

    if "nc" not in _CACHE:
        _CACHE["nc"] = _build_nc()
    nc = _CACHE["nc"]

    in_maps = _host_inputs(positions, hidden_states, w_qkv, w_o)
    res = run_bass_kernel_spmd(nc, in_maps, list(range(NCORES)))

    acc = np.zeros((S, HID), dtype=np.float32)
    for c in range(NCORES):
        acc += res.results[c]["out"]
    return acc.reshape(1, S, HID)

